# revision 11
# baseline (speedup 1.0000x reference)
"""AKT (attentive knowledge tracing) forward pass on 8 TRN2 NeuronCores.

Sharding: batch b = core//2 across 4 core-pairs; within a pair, the 8 heads
of each of the 3 MHA blocks are split 4+4 (core%2).  Pairwise AllReduces
merge the head-partial wO outputs of mha-q and mha-k; the mha-r output is
only ever consumed through Wd[:D], so wO_r @ wd_o is folded per head on the
host (rank-1 AV) and a final [1,S] logit AllReduce merges the pair.

Host preps conceptT (the Qm gather), rhs2 (=[cn, cn*correct]), and
tdt = theta^2 * (t_i - t_j) per phase, so the device never gathers or
builds dt.  Device-side per core:
  xT = cemb2^T conceptT ; yT = femb2^T conceptT + r0/dr x rhs2
  phase q (4 heads), AR(x) overlapped with phase k (4 heads), AR(y)
  overlapped with phase r's scores+softmax (which need only x_hat),
  then rank-1 AV against y_hat, logit AR, sigmoid.

Per-block softmax chain engine split:
  ACT: e=exp(sc), f=exp(arg), e2=exp(s)+accum
  DVE: cs=scan(e), recips, nd=cs/r-1, s=sc*f
  GPS: arg=nd*tdt, A=e2/r2
"""

import os
import numpy as np
import ml_dtypes

import concourse.bass as bass
import concourse.mybir as mybir
from concourse import bacc, tile
from concourse.bass_utils import run_bass_kernel_spmd

F32 = mybir.dt.float32
BF16 = mybir.dt.bfloat16
AF = mybir.ActivationFunctionType
OP = mybir.AluOpType

B, S, P, C, D, H = 4, 512, 5000, 256, 256, 8
NB = S // 128           # 4 row blocks
ND = D // 128           # 2 chunks of D
HPC = H // 2            # heads per core
N_CORES = 8
NEG = -30000.0

# blob layout (bf16 [128, 1536])
BL_MASKNS = 0
BL_MASKS = 128
BL_I128 = 256
BL_CEMB = 384
BL_FEMB = 896
BL_WDX = 1408
# misc layout (bf16 [1, 2048]): r0 | dr | cn | ccn | bd(f32 as 2 slots)
MI_R0, MI_DR, MI_CN, MI_CCN, MI_BD = 0, 256, 512, 1024, 1536


def build_kernel(debug=False):
    nc = bacc.Bacc(None, target_bir_lowering=False, debug=False, num_devices=N_CORES)

    dp = lambda name, shape, dt: nc.declare_dram_parameter(name, shape, dt, isOutput=False)
    ct2_d = dp("ct2", [128, ND * S], BF16)
    blob_d = dp("blob", [128, 1536], BF16)
    misc_d = dp("misc", [1, 2048], BF16)
    tdt_d = dp("tdt", [128, 3 * NB, S], BF16)
    qw_d = dp("qw", [128, 2 * HPC * 512], BF16)    # m chunks then wu chunks
    kw_d = dp("kw", [128, 2 * HPC * 512], BF16)
    rw_d = dp("rw", [128, HPC * 512 + HPC * ND], BF16)  # m chunks then uvec chunks

    out_d = nc.declare_dram_parameter("out", [1, S], F32, isOutput=True)
    dbg_d = {}
    if debug:
        for name in ("xT", "yT"):
            dbg_d[name] = nc.declare_dram_parameter("dbg_" + name, [D, S], BF16, isOutput=True)
        for name in ("xhatT", "yhatT"):
            dbg_d[name] = nc.declare_dram_parameter("dbg_" + name, [D, S], BF16, isOutput=True)

    from contextlib import ExitStack
    with tile.TileContext(nc) as tc, ExitStack() as es:
        pp_o = es.enter_context(tc.tile_pool(name="pp_o", bufs=2, space="PSUM"))
        pp_sc = es.enter_context(tc.tile_pool(name="pp_sc", bufs=3, space="PSUM"))
        pp_w = es.enter_context(tc.tile_pool(name="pp_w", bufs=3, space="PSUM"))
        wk = es.enter_context(tc.tile_pool(name="wk", bufs=3))
        hp = es.enter_context(tc.tile_pool(name="hp", bufs=3))
        pers = es.enter_context(tc.tile_pool(name="pers", bufs=1))
        dram = es.enter_context(tc.tile_pool(name="dram", bufs=2, space="DRAM"))

        pt = lambda shape, dt, name: pers.tile(shape, dt, name=name, tag=name)

        # ---------------- persistent SBUF + loads ---------------------------
        cT = pt([128, ND, S], BF16, "cT")
        blob = pt([128, 1536], BF16, "blob")
        misc = pt([1, 2048], BF16, "misc")
        tdt = pt([128, 3 * NB, S], BF16, "tdt")
        qw = pt([128, 2 * HPC, 512], BF16, "qw")
        kw = pt([128, 2 * HPC, 512], BF16, "kw")
        rwm = pt([128, HPC, 512], BF16, "rwm")
        rwu = pt([128, HPC, ND], BF16, "rwu")

        xT = [pt([128, S], BF16, f"xT{a}") for a in range(ND)]
        yT = [pt([128, S], BF16, f"yT{a}") for a in range(ND)]
        xhT = [pt([128, S], BF16, f"xhT{a}") for a in range(ND)]
        yhT = [pt([128, S], BF16, f"yhT{a}") for a in range(ND)]

        # upfront DMAs: sync + vector queues, first-needed first
        nc.sync.dma_start(cT[:].rearrange("p a s -> p (a s)"), ct2_d[:])
        nc.scalar.dma_start(blob[:], blob_d[:])
        nc.scalar.dma_start(misc[:], misc_d[:])
        nc.sync.dma_start(qw[:].rearrange("p a s -> p (a s)"), qw_d[:])
        nc.scalar.dma_start(tdt[:].rearrange("p a s -> p (a s)"), tdt_d[:])
        nc.sync.dma_start(kw[:].rearrange("p a s -> p (a s)"), kw_d[:])
        nc.scalar.dma_start(rwm[:].rearrange("p a s -> p (a s)"), rw_d[:, :HPC * 512])
        nc.scalar.dma_start(rwu[:].rearrange("p a s -> p (a s)"), rw_d[:, HPC * 512:])

        maskns = blob[:, BL_MASKNS:BL_MASKNS + 128]
        masks = blob[:, BL_MASKS:BL_MASKS + 128]
        i128b = blob[:, BL_I128:BL_I128 + 128]
        cemb = blob[:, BL_CEMB:BL_CEMB + 512]
        femb = blob[:, BL_FEMB:BL_FEMB + 512]
        wdx = blob[:, BL_WDX:BL_WDX + ND]
        r0v = misc[:, MI_R0:MI_R0 + D]
        drv = misc[:, MI_DR:MI_DR + D]
        cnr = misc[:, MI_CN:MI_CN + S]
        ccnr = misc[:, MI_CCN:MI_CCN + S]
        bdv = misc[:, MI_BD:MI_BD + 2].bitcast(F32)

        # ---------------- embedding ----------------------------------------
        for ec in range(ND):
            x_ps = pp_w.tile([128, S], F32, name="x_ps", tag="w")
            for a in range(ND):
                nc.tensor.matmul(x_ps[:], cemb[:, a * D + 128 * ec: a * D + 128 * (ec + 1)],
                                 cT[:, a, :], start=(a == 0), stop=(a == ND - 1))
            nc.scalar.copy(xT[ec][:], x_ps[:])
            y_ps = pp_w.tile([128, S], F32, name="y_ps", tag="w")
            for a in range(ND):
                nc.tensor.matmul(y_ps[:], femb[:, a * D + 128 * ec: a * D + 128 * (ec + 1)],
                                 cT[:, a, :], start=(a == 0), stop=False)
            nc.tensor.matmul(y_ps[:], r0v[:, 128 * ec:128 * (ec + 1)], cnr[:],
                             start=False, stop=False)
            nc.tensor.matmul(y_ps[:], drv[:, 128 * ec:128 * (ec + 1)], ccnr[:],
                             start=False, stop=True)
            nc.vector.tensor_copy(yT[ec][:], y_ps[:])
        if debug:
            for a in range(ND):
                nc.sync.dma_start(dbg_d["xT"][128 * a:128 * (a + 1), :], xT[a][:])
                nc.sync.dma_start(dbg_d["yT"][128 * a:128 * (a + 1), :], yT[a][:])

        # ---------------- softmax chain for one row-block -------------------
        def chain(sc_ps, tdt_row, J, strict0, a_dst):
            e = wk.tile([128, S], F32, name="e", tag="e")
            nc.scalar.activation(e[:, :J], sc_ps[:, :J], AF.Exp)
            cs = wk.tile([128, S], F32, name="cs", tag="cs")
            nc.vector.tensor_tensor_scan(cs[:, :J], e[:, :J], e[:, :J],
                                         0.0, OP.add, OP.bypass)
            rec = wk.tile([128, 1], F32, name="rec", tag="rec")
            if strict0:
                rr = wk.tile([128, 1], F32, name="rr", tag="rr")
                nc.vector.tensor_scalar_max(rr[:], cs[:, J - 1:J], 1e-30)
                nc.vector.reciprocal(rec[:], rr[:])
            else:
                nc.vector.reciprocal(rec[:], cs[:, J - 1:J])
            nd = wk.tile([128, S], BF16, name="nd", tag="nd")
            nc.vector.tensor_scalar(nd[:, :J], cs[:, :J], rec[:], -1.0, OP.mult, OP.add)
            arg = wk.tile([128, S], BF16, name="arg", tag="arg")
            nc.gpsimd.tensor_mul(arg[:, :J], nd[:, :J], tdt_row[:, :J])
            f = wk.tile([128, S], BF16, name="f", tag="f")
            nc.scalar.activation(f[:, :J], arg[:, :J], AF.Exp)
            s = wk.tile([128, S], BF16, name="s", tag="s")
            nc.vector.tensor_mul(s[:, :J], sc_ps[:, :J], f[:, :J])
            e2 = wk.tile([128, S], BF16, name="e2", tag="e2")
            r2 = wk.tile([128, 1], F32, name="r2", tag="r2")
            nc.scalar.activation(e2[:, :J], s[:, :J], AF.Exp, accum_out=r2[:])
            rec2 = wk.tile([128, 1], F32, name="rec2", tag="rec2")
            if strict0:
                nc.vector.tensor_scalar_max(r2[:], r2[:], 1e-30)
            nc.vector.reciprocal(rec2[:], r2[:])
            nc.gpsimd.tensor_scalar_mul(a_dst[:, :J], e2[:, :J], rec2[:])

        def scores_block(gt_sb, qxT, ib, mask):
            J = 128 * (ib + 1)
            sc_ps = pp_sc.tile([128, S], F32, name="sc_ps", tag="sc")
            for a in range(ND):
                nc.tensor.matmul(sc_ps[:, :J], gt_sb[a][:, 128 * ib:128 * (ib + 1)],
                                 qxT[a][:, :J], start=(a == 0), stop=False)
            nc.tensor.matmul(sc_ps[:, 128 * ib:J], i128b, mask, start=False, stop=True)
            return sc_ps, J

        def gt_head(msl, qxT):
            """G^T = M^T qx^T, [ND] bf16 [128,S] tiles.  msl: [128,512] M chunks."""
            gt_sb = [hp.tile([128, S], BF16, name=f"gt{ec}", tag=f"gt{ec}")
                     for ec in range(ND)]
            for ec in range(ND):
                gt_ps = pp_w.tile([128, S], F32, name="gt_ps", tag="w")
                for a in range(ND):
                    nc.tensor.matmul(gt_ps[:],
                                     msl[:, a * D + 128 * ec: a * D + 128 * (ec + 1)],
                                     qxT[a][:], start=(a == 0), stop=(a == ND - 1))
                if ec == 0:
                    nc.scalar.copy(gt_sb[ec][:], gt_ps[:])
                else:
                    nc.vector.tensor_copy(gt_sb[ec][:], gt_ps[:])
            return gt_sb

        # ---------------- full MHA phase (q / k) ----------------------------
        def mha_phase(wsl, qxT, vxT, tdt_base):
            o_ps = [pp_o.tile([128, S], F32, name=f"o_ps{ec}", tag="o")
                    for ec in range(ND)]
            for h in range(HPC):
                # WV[j, e] = sum_d vx[j, d] wU[d, e]
                wu = wsl[:, HPC + h, :]
                wv_sb = [hp.tile([128, S], BF16, name=f"wv{q_}", tag=f"wv{q_}")
                         for q_ in range(2)]
                for half in range(2):
                    wv_ps = pp_w.tile([128, S], F32, name="wv_ps", tag="w")
                    for j2 in range(2):
                        jb = 2 * half + j2
                        for a in range(ND):
                            nc.tensor.matmul(
                                wv_ps[:, 256 * j2:256 * (j2 + 1)],
                                vxT[a][:, 128 * jb:128 * (jb + 1)],
                                wu[:, a * D:(a + 1) * D],
                                start=(a == 0), stop=(a == ND - 1))
                    if half == 0:
                        nc.scalar.copy(wv_sb[half][:], wv_ps[:])
                    else:
                        nc.vector.tensor_copy(wv_sb[half][:], wv_ps[:])

                gt_sb = gt_head(wsl[:, h, :], qxT)

                a_full = hp.tile([128, NB, S], BF16, name="a_full", tag="af")
                if os.environ.get("AKT_SIM"):
                    nc.gpsimd.memset(a_full[:], 0)
                for ib in range(NB):
                    sc_ps, J = scores_block(gt_sb, qxT, ib, maskns)
                    chain(sc_ps, tdt[:, tdt_base + ib, :], J, False, a_full[:, ib, :])

                e2t = hp.tile([128, NB * NB, 128], BF16, name="e2t", tag="e2t")
                nc.sync.dma_start_transpose(e2t[:], a_full[:].rearrange("p a j -> p (a j)"))
                for ec in range(ND):
                    for jb in range(NB):
                        rhs = e2t[:, NB * jb + jb:NB * NB:NB, :]
                        lhsT = wv_sb[jb // 2][:, 256 * (jb % 2) + 128 * ec:
                                              256 * (jb % 2) + 128 * (ec + 1)]
                        nc.tensor.matmul(o_ps[ec][:, 128 * jb:], lhsT, rhs,
                                         start=(h == 0 and jb == 0),
                                         stop=(h == HPC - 1 and jb == NB - 1),
                                         skip_group_check=True)
            return o_ps

        def reduce_pair(o_ps, dstT, name, copy_engines=("scalar", "vector")):
            """o_ps [ND] psum -> bf16 -> DRAM -> pair AllReduce -> dstT tiles."""
            part = [pt([128, S], BF16, f"{name}p{a}") for a in range(ND)]
            for a in range(ND):
                if copy_engines[a % 2] == "scalar":
                    nc.scalar.copy(part[a][:], o_ps[a][:])
                else:
                    nc.vector.tensor_copy(part[a][:], o_ps[a][:])
            bnc = dram.tile([ND * 128, S], BF16, name=f"bnc_{name}")
            bnco = dram.tile([ND * 128, S], BF16, name=f"bnco_{name}")
            for a in range(ND):
                nc.sync.dma_start(bnc[128 * a:128 * (a + 1), :], part[a][:])
            nc.gpsimd.collective_compute(
                "AllReduce", OP.add,
                replica_groups=[[0, 1], [2, 3], [4, 5], [6, 7]],
                ins=[bnc.opt()], outs=[bnco.opt()])
            for a in range(ND):
                nc.sync.dma_start(dstT[a][:], bnco[128 * a:128 * (a + 1), :])

        # ---------------- phase q, AR(x) over phase k, AR(y) ----------------
        o_q = mha_phase(qw, xT, xT, 0)
        reduce_pair(o_q, xhT, "x")
        o_k = mha_phase(kw, yT, yT, NB)
        reduce_pair(o_k, yhT, "y", copy_engines=("vector", "scalar"))
        if debug:
            for a in range(ND):
                nc.sync.dma_start(dbg_d["xhatT"][128 * a:128 * (a + 1), :], xhT[a][:])
                nc.sync.dma_start(dbg_d["yhatT"][128 * a:128 * (a + 1), :], yhT[a][:])

        # ---------------- phase r: scores+softmax on x_hat only -------------
        e2t_r = [pt([128, NB * NB, 128], BF16, f"e2tr{h}") for h in range(HPC)]
        for h in range(HPC):
            gt_sb = gt_head(rwm[:, h, :], xhT)
            a_full = hp.tile([128, NB, S], BF16, name="a_full", tag="af")
            if os.environ.get("AKT_SIM"):
                nc.gpsimd.memset(a_full[:], 0)
            for ib in range(NB):
                sc_ps, J = scores_block(gt_sb, xhT, ib, masks)
                chain(sc_ps, tdt[:, 2 * NB + ib, :], J, ib == 0, a_full[:, ib, :])
            nc.sync.dma_start_transpose(e2t_r[h][:],
                                        a_full[:].rearrange("p a j -> p (a j)"))

        # ---------------- phase r tail: rank-1 AV against y_hat -------------
        lgx_ps = pp_o.tile([1, S], F32, name="lgx_ps", tag="o")
        for a in range(ND):
            nc.tensor.matmul(lgx_ps[:], wdx[:, a:a + 1], xhT[a][:],
                             start=(a == 0), stop=(a == ND - 1))
        o3_ps = pp_o.tile([1, S], F32, name="o3_ps", tag="o")
        for h in range(HPC):
            wvl_ps = pp_w.tile([128, NB], F32, name="wvl_ps", tag="w")
            for jb in range(NB):
                for a in range(ND):
                    nc.tensor.matmul(wvl_ps[:, jb:jb + 1],
                                     yhT[a][:, 128 * jb:128 * (jb + 1)],
                                     rwu[:, h, a:a + 1],
                                     start=(a == 0), stop=(a == ND - 1))
            wvl = hp.tile([128, NB], BF16, name="wvl", tag="wvl")
            nc.vector.tensor_copy(wvl[:], wvl_ps[:])
            for jb in range(NB):
                for ib in range(jb, NB):
                    nc.tensor.matmul(o3_ps[0:1, 128 * ib:128 * (ib + 1)],
                                     wvl[:, jb:jb + 1],
                                     e2t_r[h][:, NB * ib + jb, :],
                                     start=(h == 0 and jb == 0 and ib == 0),
                                     stop=(h == HPC - 1 and jb == NB - 1 and ib == NB - 1),
                                     skip_group_check=True)

        # ---------------- logit AR + sigmoid --------------------------------
        lg3 = pt([1, S], F32, "lg3")
        nc.vector.tensor_copy(lg3[:], o3_ps[:])
        nc.gpsimd.memset(lg3[:, 0:1], 0)
        bnc2 = dram.tile([1, S], F32, name="bnc2")
        bnc2o = dram.tile([1, S], F32, name="bnc2o")
        nc.sync.dma_start(bnc2[:], lg3[:])
        nc.gpsimd.collective_compute(
            "AllReduce", OP.add,
            replica_groups=[[0, 1], [2, 3], [4, 5], [6, 7]],
            ins=[bnc2.opt()], outs=[bnc2o.opt()])
        lg3o = pt([1, S], F32, "lg3o")
        nc.sync.dma_start(lg3o[:], bnc2o[:])
        logit = pt([1, S], F32, "logit")
        nc.vector.tensor_add(logit[:], lgx_ps[:], lg3o[:])
        pred = pt([1, S], F32, "pred")
        nc.scalar.activation(pred[:], logit[:], AF.Sigmoid, bias=bdv)
        nc.sync.dma_start(out_d[:], pred[:])

    nc.finalize()
    return nc


# ---------------------------------------------------------------------------
_NC_CACHE = {}


def _get_nc(debug=False):
    if debug not in _NC_CACHE:
        _NC_CACHE[debug] = build_kernel(debug)
    return _NC_CACHE[debug]


def _chunked(w):
    """[n*128, M] -> [128, n*M] (chunk-major columns)."""
    n = w.shape[0] // 128
    return np.ascontiguousarray(
        w.reshape(n, 128, w.shape[1]).transpose(1, 0, 2).reshape(128, n * w.shape[1]))


def _prep_core_inputs(b, g, item, timestamp, correct, Qm, c_embed, d_embed, f_embed,
                      mu_q, r_embed, Wd, bd, weights):
    f32 = np.float32
    bf = ml_dtypes.bfloat16
    it = item[b].astype(np.int64) - 1
    valid = it >= 0
    concept = np.where(valid[:, None], Qm[np.clip(it, 0, None)].astype(f32), 0.0)  # [S,C]
    ct2 = _chunked(np.ascontiguousarray(concept.T)).astype(bf)                      # [128, 2*S]
    cn = concept.sum(1)
    ccn = cn * correct[b].astype(f32)

    ts = timestamp[b].astype(np.float64)
    dtm = ts[:, None] - ts[None, :]                       # [S, S]
    tdt = np.zeros((128, 3 * NB, S), f32)
    for pi, p in enumerate("qkr"):
        th2 = float(np.asarray(weights[p + "_theta"], np.float64)[0, 0]) ** 2
        for ib in range(NB):
            tdt[:, pi * NB + ib, :] = (th2 * dtm[128 * ib:128 * (ib + 1), :]).astype(f32)

    r, c = np.mgrid[0:128, 0:128]
    blob = np.zeros((128, 1536), f32)
    blob[:, BL_MASKNS:BL_MASKNS + 128] = np.where(c <= r, 0.0, NEG)
    blob[:, BL_MASKS:BL_MASKS + 128] = np.where(c < r, 0.0, NEG)
    blob[:, BL_I128:BL_I128 + 128] = np.eye(128, dtype=f32)
    blob[:, BL_CEMB:BL_CEMB + 512] = _chunked((mu_q * d_embed + c_embed).astype(f32))
    blob[:, BL_FEMB:BL_FEMB + 512] = _chunked((mu_q * f_embed).astype(f32))
    blob[:, BL_WDX:BL_WDX + ND] = Wd[D:2 * D].reshape(ND, 128).T
    misc = np.zeros((1, 2048), f32)
    misc[0, MI_R0:MI_R0 + D] = r_embed[0]
    misc[0, MI_DR:MI_DR + D] = r_embed[1] - r_embed[0]
    misc[0, MI_CN:MI_CN + S] = cn
    misc[0, MI_CCN:MI_CCN + S] = ccn
    misc_bf = misc.astype(bf)
    misc_bf[0, MI_BD:MI_BD + 2] = (
        np.asarray(bd, f32).reshape(-1)[:1].view(np.uint16).view(bf))

    hs = range(HPC * g, HPC * g + HPC)

    def phase_w(p):
        wQ, wK, wV, wO = (weights[p + "_wQ"], weights[p + "_wK"],
                          weights[p + "_wV"], weights[p + "_wO"])
        Ms = [_chunked((wQ[h] @ wK[h].T / np.sqrt(f32(D))).astype(f32)) for h in hs]
        Us = [(wV[h] @ wO[h * D:(h + 1) * D]).astype(f32) for h in hs]
        return Ms, Us

    qM, qU = phase_w("q")
    kM, kU = phase_w("k")
    rM, rU = phase_w("r")
    wdo = Wd[:D].reshape(D, 1).astype(f32)
    qw = np.concatenate(qM + [_chunked(u) for u in qU], axis=1)
    kw = np.concatenate(kM + [_chunked(u) for u in kU], axis=1)
    ruv = [_chunked(u @ wdo) for u in rU]                  # each [128, 2]
    rw = np.concatenate(rM + ruv, axis=1)

    return {
        "ct2": ct2,
        "blob": blob.astype(bf),
        "misc": misc_bf,
        "tdt": tdt.reshape(128, 3 * NB, S).astype(bf),
        "qw": qw.astype(bf),
        "kw": kw.astype(bf),
        "rw": rw.astype(bf),
    }


LAST_RESULTS = [None]


def kernel(item, timestamp, correct, Qm, c_embed, d_embed, f_embed, mu_q,
           r_embed, Wd, bd, q_wQ, q_wK, q_wV, q_wO, q_theta,
           k_wQ, k_wK, k_wV, k_wO, k_theta, r_wQ, r_wK, r_wV, r_wO, r_theta,
           _debug=False, _trace=False):
    weights = {
        "q_wQ": q_wQ, "q_wK": q_wK, "q_wV": q_wV, "q_wO": q_wO, "q_theta": q_theta,
        "k_wQ": k_wQ, "k_wK": k_wK, "k_wV": k_wV, "k_wO": k_wO, "k_theta": k_theta,
        "r_wQ": r_wQ, "r_wK": r_wK, "r_wV": r_wV, "r_wO": r_wO, "r_theta": r_theta,
    }
    weights = {k: np.asarray(v) for k, v in weights.items()}
    args = (np.asarray(item), np.asarray(timestamp), np.asarray(correct),
            np.asarray(Qm), np.asarray(c_embed), np.asarray(d_embed),
            np.asarray(f_embed), np.asarray(mu_q), np.asarray(r_embed),
            np.asarray(Wd), np.asarray(bd))
    in_maps = []
    for core in range(N_CORES):
        b, g = core // 2, core % 2
        in_maps.append(_prep_core_inputs(b, g, *args, weights))
    nc = _get_nc(_debug)
    res = run_bass_kernel_spmd(nc, in_maps, core_ids=list(range(N_CORES)),
                               trace=_trace,
                               trace_cores=list(range(N_CORES)) if _trace == "all" else None)
    LAST_RESULTS[0] = res
    outs = res.results
    pred = np.zeros((B, S, 1), np.float32)
    for b in range(B):
        pred[b, :, 0] = outs[2 * b]["out"][0]
    if _debug:
        return pred, outs
    return pred


# revision 12
# speedup vs baseline: 1.9957x; 1.9957x over previous
"""AKT (attentive knowledge tracing) forward pass on 8 TRN2 NeuronCores.

Sharding: batch b = core//2 across 4 core-pairs; within a pair, the 8 heads
of each of the 3 MHA blocks are split 4+4 (core%2).  Pairwise AllReduces
merge the head-partial wO outputs of mha-q and mha-k; the mha-r output is
only ever consumed through Wd[:D], so wO_r @ wd_o is folded per head on the
host (rank-1 AV) and a final [1,S] logit AllReduce merges the pair.

Host preps conceptT (the Qm gather), rhs2 (=[cn, cn*correct]), and
tdt = theta^2 * (t_i - t_j) per phase, so the device never gathers or
builds dt.  Device-side per core:
  xT = cemb2^T conceptT ; yT = femb2^T conceptT + r0/dr x rhs2
  phase q (4 heads), AR(x) overlapped with phase k (4 heads), AR(y)
  overlapped with phase r's scores+softmax (which need only x_hat),
  then rank-1 AV against y_hat, logit AR, sigmoid.

Per-block softmax chain engine split:
  ACT: e=exp(sc), f=exp(arg), e2=exp(s)+accum
  DVE: cs=scan(e), recips, nd=cs/r-1, s=sc*f
  GPS: arg=nd*tdt, A=e2/r2
"""

import os
import numpy as np
import ml_dtypes

import concourse.bass as bass
import concourse.mybir as mybir
from concourse import bacc, tile
from concourse.bass_utils import run_bass_kernel_spmd

F32 = mybir.dt.float32
BF16 = mybir.dt.bfloat16
AF = mybir.ActivationFunctionType
OP = mybir.AluOpType

B, S, P, C, D, H = 4, 512, 5000, 256, 256, 8
NB = S // 128           # 4 row blocks
ND = D // 128           # 2 chunks of D
HPC = H // 2            # heads per core
N_CORES = 8
NEG = -30000.0

# blob layout (bf16 [128, 1536])
BL_MASKNS = 0
BL_MASKS = 128
BL_I128 = 256
BL_CEMB = 384
BL_FEMB = 896
BL_WDX = 1408
# misc layout (bf16 [1, 2048]): r0 | dr | cn | ccn | bd(f32 as 2 slots)
MI_R0, MI_DR, MI_CN, MI_CCN, MI_BD = 0, 256, 512, 1024, 1536


def build_kernel(debug=False):
    nc = bacc.Bacc(None, target_bir_lowering=False, debug=False, num_devices=N_CORES)

    dp = lambda name, shape, dt: nc.declare_dram_parameter(name, shape, dt, isOutput=False)
    ct2_d = dp("ct2", [128, ND * S], BF16)
    blob_d = dp("blob", [128, 1536], BF16)
    misc_d = dp("misc", [1, 2048], BF16)
    tdt_d = dp("tdt", [128, 3 * NB, S], BF16)
    qw_d = dp("qw", [128, 2 * HPC * 512], BF16)    # m chunks then wu chunks
    kw_d = dp("kw", [128, 2 * HPC * 512], BF16)
    rw_d = dp("rw", [128, HPC * 512 + HPC * ND], BF16)  # m chunks then uvec chunks

    out_d = nc.declare_dram_parameter("out", [1, S], F32, isOutput=True)
    dbg_d = {}
    if debug:
        for name in ("xT", "yT"):
            dbg_d[name] = nc.declare_dram_parameter("dbg_" + name, [D, S], BF16, isOutput=True)
        for name in ("xhatT", "yhatT"):
            dbg_d[name] = nc.declare_dram_parameter("dbg_" + name, [D, S], BF16, isOutput=True)

    from contextlib import ExitStack
    with tile.TileContext(nc) as tc, ExitStack() as es:
        pp_o = es.enter_context(tc.tile_pool(name="pp_o", bufs=2, space="PSUM"))
        pp_sc = es.enter_context(tc.tile_pool(name="pp_sc", bufs=3, space="PSUM"))
        pp_w = es.enter_context(tc.tile_pool(name="pp_w", bufs=3, space="PSUM"))
        wk = es.enter_context(tc.tile_pool(name="wk", bufs=3))
        hp = es.enter_context(tc.tile_pool(name="hp", bufs=3))
        pers = es.enter_context(tc.tile_pool(name="pers", bufs=1))
        dram = es.enter_context(tc.tile_pool(name="dram", bufs=2, space="DRAM"))

        pt = lambda shape, dt, name: pers.tile(shape, dt, name=name, tag=name)

        # ---------------- persistent SBUF + loads ---------------------------
        cT = pt([128, ND, S], BF16, "cT")
        blob = pt([128, 1536], BF16, "blob")
        misc = pt([1, 2048], BF16, "misc")
        tdt = pt([128, 3 * NB, S], BF16, "tdt")
        qw = pt([128, 2 * HPC, 512], BF16, "qw")
        kw = pt([128, 2 * HPC, 512], BF16, "kw")
        rwm = pt([128, HPC, 512], BF16, "rwm")
        rwu = pt([128, HPC, ND], BF16, "rwu")

        xT = [pt([128, S], BF16, f"xT{a}") for a in range(ND)]
        yT = [pt([128, S], BF16, f"yT{a}") for a in range(ND)]
        xhT = [pt([128, S], BF16, f"xhT{a}") for a in range(ND)]
        yhT = [pt([128, S], BF16, f"yhT{a}") for a in range(ND)]

        # upfront DMAs: sync + vector queues, first-needed first
        nc.sync.dma_start(cT[:].rearrange("p a s -> p (a s)"), ct2_d[:])
        nc.scalar.dma_start(blob[:], blob_d[:])
        nc.scalar.dma_start(misc[:], misc_d[:])
        nc.sync.dma_start(qw[:].rearrange("p a s -> p (a s)"), qw_d[:])
        nc.scalar.dma_start(tdt[:].rearrange("p a s -> p (a s)"), tdt_d[:])
        nc.sync.dma_start(kw[:].rearrange("p a s -> p (a s)"), kw_d[:])
        nc.scalar.dma_start(rwm[:].rearrange("p a s -> p (a s)"), rw_d[:, :HPC * 512])
        nc.scalar.dma_start(rwu[:].rearrange("p a s -> p (a s)"), rw_d[:, HPC * 512:])

        maskns = blob[:, BL_MASKNS:BL_MASKNS + 128]
        masks = blob[:, BL_MASKS:BL_MASKS + 128]
        i128b = blob[:, BL_I128:BL_I128 + 128]
        cemb = blob[:, BL_CEMB:BL_CEMB + 512]
        femb = blob[:, BL_FEMB:BL_FEMB + 512]
        wdx = blob[:, BL_WDX:BL_WDX + ND]
        r0v = misc[:, MI_R0:MI_R0 + D]
        drv = misc[:, MI_DR:MI_DR + D]
        cnr = misc[:, MI_CN:MI_CN + S]
        ccnr = misc[:, MI_CCN:MI_CCN + S]
        bdv = misc[:, MI_BD:MI_BD + 2].bitcast(F32)

        # ---------------- embedding ----------------------------------------
        for ec in range(ND):
            x_ps = pp_w.tile([128, S], F32, name="x_ps", tag="w")
            for a in range(ND):
                nc.tensor.matmul(x_ps[:], cemb[:, a * D + 128 * ec: a * D + 128 * (ec + 1)],
                                 cT[:, a, :], start=(a == 0), stop=(a == ND - 1))
            nc.scalar.copy(xT[ec][:], x_ps[:])
            y_ps = pp_w.tile([128, S], F32, name="y_ps", tag="w")
            for a in range(ND):
                nc.tensor.matmul(y_ps[:], femb[:, a * D + 128 * ec: a * D + 128 * (ec + 1)],
                                 cT[:, a, :], start=(a == 0), stop=False)
            nc.tensor.matmul(y_ps[:], r0v[:, 128 * ec:128 * (ec + 1)], cnr[:],
                             start=False, stop=False)
            nc.tensor.matmul(y_ps[:], drv[:, 128 * ec:128 * (ec + 1)], ccnr[:],
                             start=False, stop=True)
            nc.vector.tensor_copy(yT[ec][:], y_ps[:])
        if debug:
            for a in range(ND):
                nc.sync.dma_start(dbg_d["xT"][128 * a:128 * (a + 1), :], xT[a][:])
                nc.sync.dma_start(dbg_d["yT"][128 * a:128 * (a + 1), :], yT[a][:])

        # ---------------- softmax chain for one row-block -------------------
        def chain(sc_ps, tdt_row, J, strict0, a_dst):
            e = wk.tile([128, S], BF16, name="e", tag="e")
            nc.scalar.activation(e[:, :J], sc_ps[:, :J], AF.Exp)
            cs = wk.tile([128, S], BF16, name="cs", tag="cs")
            nc.vector.tensor_tensor_scan(cs[:, :J], e[:, :J], e[:, :J],
                                         0.0, OP.add, OP.bypass)
            rec = wk.tile([128, 1], F32, name="rec", tag="rec")
            if strict0:
                rr = wk.tile([128, 1], F32, name="rr", tag="rr")
                nc.vector.tensor_scalar_max(rr[:], cs[:, J - 1:J], 1e-30)
                nc.vector.reciprocal(rec[:], rr[:])
            else:
                nc.vector.reciprocal(rec[:], cs[:, J - 1:J])
            nd = wk.tile([128, S], BF16, name="nd", tag="nd")
            nc.vector.tensor_scalar(nd[:, :J], cs[:, :J], rec[:], -1.0, OP.mult, OP.add)
            arg = wk.tile([128, S], BF16, name="arg", tag="arg")
            nc.gpsimd.tensor_mul(arg[:, :J], nd[:, :J], tdt_row[:, :J])
            f = wk.tile([128, S], BF16, name="f", tag="f")
            nc.scalar.activation(f[:, :J], arg[:, :J], AF.Exp)
            s = wk.tile([128, S], BF16, name="s", tag="s")
            nc.vector.tensor_mul(s[:, :J], sc_ps[:, :J], f[:, :J])
            e2 = wk.tile([128, S], BF16, name="e2", tag="e2")
            r2 = wk.tile([128, 1], F32, name="r2", tag="r2")
            nc.scalar.activation(e2[:, :J], s[:, :J], AF.Exp, accum_out=r2[:])
            rec2 = wk.tile([128, 1], F32, name="rec2", tag="rec2")
            if strict0:
                nc.vector.tensor_scalar_max(r2[:], r2[:], 1e-30)
            nc.vector.reciprocal(rec2[:], r2[:])
            nc.vector.tensor_scalar_mul(a_dst[:, :J], e2[:, :J], rec2[:])

        def scores_block(gt_sb, qxT, ib, mask):
            J = 128 * (ib + 1)
            sc_ps = pp_sc.tile([128, S], F32, name="sc_ps", tag="sc")
            for a in range(ND):
                nc.tensor.matmul(sc_ps[:, :J], gt_sb[a][:, 128 * ib:128 * (ib + 1)],
                                 qxT[a][:, :J], start=(a == 0), stop=False)
            nc.tensor.matmul(sc_ps[:, 128 * ib:J], i128b, mask, start=False, stop=True)
            return sc_ps, J

        def gt_head(msl, qxT):
            """G^T = M^T qx^T, [ND] bf16 [128,S] tiles.  msl: [128,512] M chunks."""
            gt_sb = [hp.tile([128, S], BF16, name=f"gt{ec}", tag=f"gt{ec}")
                     for ec in range(ND)]
            for ec in range(ND):
                gt_ps = pp_w.tile([128, S], F32, name="gt_ps", tag="w")
                for a in range(ND):
                    nc.tensor.matmul(gt_ps[:],
                                     msl[:, a * D + 128 * ec: a * D + 128 * (ec + 1)],
                                     qxT[a][:], start=(a == 0), stop=(a == ND - 1))
                if ec == 0:
                    nc.scalar.copy(gt_sb[ec][:], gt_ps[:])
                else:
                    nc.vector.tensor_copy(gt_sb[ec][:], gt_ps[:])
            return gt_sb

        # ---------------- full MHA phase (q / k) ----------------------------
        def mha_phase(wsl, qxT, vxT, tdt_base):
            o_ps = [pp_o.tile([128, S], F32, name=f"o_ps{ec}", tag="o")
                    for ec in range(ND)]
            for h in range(HPC):
                # WV[j, e] = sum_d vx[j, d] wU[d, e]
                wu = wsl[:, HPC + h, :]
                wv_sb = [hp.tile([128, S], BF16, name=f"wv{q_}", tag=f"wv{q_}")
                         for q_ in range(2)]
                for half in range(2):
                    wv_ps = pp_w.tile([128, S], F32, name="wv_ps", tag="w")
                    for j2 in range(2):
                        jb = 2 * half + j2
                        for a in range(ND):
                            nc.tensor.matmul(
                                wv_ps[:, 256 * j2:256 * (j2 + 1)],
                                vxT[a][:, 128 * jb:128 * (jb + 1)],
                                wu[:, a * D:(a + 1) * D],
                                start=(a == 0), stop=(a == ND - 1))
                    if half == 0:
                        nc.scalar.copy(wv_sb[half][:], wv_ps[:])
                    else:
                        nc.vector.tensor_copy(wv_sb[half][:], wv_ps[:])

                gt_sb = gt_head(wsl[:, h, :], qxT)

                a_full = hp.tile([128, NB, S], BF16, name="a_full", tag="af")
                if os.environ.get("AKT_SIM"):
                    nc.gpsimd.memset(a_full[:], 0)
                for ib in range(NB):
                    sc_ps, J = scores_block(gt_sb, qxT, ib, maskns)
                    chain(sc_ps, tdt[:, tdt_base + ib, :], J, False, a_full[:, ib, :])

                e2t = hp.tile([128, NB * NB, 128], BF16, name="e2t", tag="e2t")
                nc.sync.dma_start_transpose(e2t[:], a_full[:].rearrange("p a j -> p (a j)"))
                for ec in range(ND):
                    for jb in range(NB):
                        rhs = e2t[:, NB * jb + jb:NB * NB:NB, :]
                        lhsT = wv_sb[jb // 2][:, 256 * (jb % 2) + 128 * ec:
                                              256 * (jb % 2) + 128 * (ec + 1)]
                        nc.tensor.matmul(o_ps[ec][:, 128 * jb:], lhsT, rhs,
                                         start=(h == 0 and jb == 0),
                                         stop=(h == HPC - 1 and jb == NB - 1),
                                         skip_group_check=True)
            return o_ps

        def reduce_pair(o_ps, dstT, name, copy_engines=("scalar", "vector")):
            """o_ps [ND] psum -> bf16 -> DRAM -> pair AllReduce -> dstT tiles."""
            part = [pt([128, S], BF16, f"{name}p{a}") for a in range(ND)]
            for a in range(ND):
                if copy_engines[a % 2] == "scalar":
                    nc.scalar.copy(part[a][:], o_ps[a][:])
                else:
                    nc.vector.tensor_copy(part[a][:], o_ps[a][:])
            bnc = dram.tile([ND * 128, S], BF16, name=f"bnc_{name}")
            bnco = dram.tile([ND * 128, S], BF16, name=f"bnco_{name}")
            for a in range(ND):
                nc.sync.dma_start(bnc[128 * a:128 * (a + 1), :], part[a][:])
            nc.gpsimd.collective_compute(
                "AllReduce", OP.add,
                replica_groups=[[0, 1], [2, 3], [4, 5], [6, 7]],
                ins=[bnc.opt()], outs=[bnco.opt()])
            for a in range(ND):
                nc.sync.dma_start(dstT[a][:], bnco[128 * a:128 * (a + 1), :])

        # ---------------- phase q, AR(x) over phase k, AR(y) ----------------
        o_q = mha_phase(qw, xT, xT, 0)
        reduce_pair(o_q, xhT, "x")
        o_k = mha_phase(kw, yT, yT, NB)
        reduce_pair(o_k, yhT, "y", copy_engines=("vector", "scalar"))
        if debug:
            for a in range(ND):
                nc.sync.dma_start(dbg_d["xhatT"][128 * a:128 * (a + 1), :], xhT[a][:])
                nc.sync.dma_start(dbg_d["yhatT"][128 * a:128 * (a + 1), :], yhT[a][:])

        # ---------------- phase r: scores+softmax on x_hat only -------------
        e2t_r = [pt([128, NB * NB, 128], BF16, f"e2tr{h}") for h in range(HPC)]
        for h in range(HPC):
            gt_sb = gt_head(rwm[:, h, :], xhT)
            a_full = hp.tile([128, NB, S], BF16, name="a_full", tag="af")
            if os.environ.get("AKT_SIM"):
                nc.gpsimd.memset(a_full[:], 0)
            for ib in range(NB):
                sc_ps, J = scores_block(gt_sb, xhT, ib, masks)
                chain(sc_ps, tdt[:, 2 * NB + ib, :], J, ib == 0, a_full[:, ib, :])
            nc.sync.dma_start_transpose(e2t_r[h][:],
                                        a_full[:].rearrange("p a j -> p (a j)"))

        # ---------------- phase r tail: rank-1 AV against y_hat -------------
        lgx_ps = pp_o.tile([1, S], F32, name="lgx_ps", tag="o")
        for a in range(ND):
            nc.tensor.matmul(lgx_ps[:], wdx[:, a:a + 1], xhT[a][:],
                             start=(a == 0), stop=(a == ND - 1))
        o3_ps = pp_o.tile([1, S], F32, name="o3_ps", tag="o")
        for h in range(HPC):
            wvl_ps = pp_w.tile([128, NB], F32, name="wvl_ps", tag="w")
            for jb in range(NB):
                for a in range(ND):
                    nc.tensor.matmul(wvl_ps[:, jb:jb + 1],
                                     yhT[a][:, 128 * jb:128 * (jb + 1)],
                                     rwu[:, h, a:a + 1],
                                     start=(a == 0), stop=(a == ND - 1))
            wvl = hp.tile([128, NB], BF16, name="wvl", tag="wvl")
            nc.vector.tensor_copy(wvl[:], wvl_ps[:])
            for jb in range(NB):
                for ib in range(jb, NB):
                    nc.tensor.matmul(o3_ps[0:1, 128 * ib:128 * (ib + 1)],
                                     wvl[:, jb:jb + 1],
                                     e2t_r[h][:, NB * ib + jb, :],
                                     start=(h == 0 and jb == 0 and ib == 0),
                                     stop=(h == HPC - 1 and jb == NB - 1 and ib == NB - 1),
                                     skip_group_check=True)

        # ---------------- logit AR + sigmoid --------------------------------
        lg3 = pt([1, S], F32, "lg3")
        nc.vector.tensor_copy(lg3[:], o3_ps[:])
        nc.gpsimd.memset(lg3[:, 0:1], 0)
        bnc2 = dram.tile([1, S], F32, name="bnc2")
        bnc2o = dram.tile([1, S], F32, name="bnc2o")
        nc.sync.dma_start(bnc2[:], lg3[:])
        nc.gpsimd.collective_compute(
            "AllReduce", OP.add,
            replica_groups=[[0, 1], [2, 3], [4, 5], [6, 7]],
            ins=[bnc2.opt()], outs=[bnc2o.opt()])
        lg3o = pt([1, S], F32, "lg3o")
        nc.sync.dma_start(lg3o[:], bnc2o[:])
        logit = pt([1, S], F32, "logit")
        nc.vector.tensor_add(logit[:], lgx_ps[:], lg3o[:])
        pred = pt([1, S], F32, "pred")
        nc.scalar.activation(pred[:], logit[:], AF.Sigmoid, bias=bdv)
        nc.sync.dma_start(out_d[:], pred[:])

    nc.finalize()
    return nc


# ---------------------------------------------------------------------------
_NC_CACHE = {}


def _get_nc(debug=False):
    if debug not in _NC_CACHE:
        _NC_CACHE[debug] = build_kernel(debug)
    return _NC_CACHE[debug]


def _chunked(w):
    """[n*128, M] -> [128, n*M] (chunk-major columns)."""
    n = w.shape[0] // 128
    return np.ascontiguousarray(
        w.reshape(n, 128, w.shape[1]).transpose(1, 0, 2).reshape(128, n * w.shape[1]))


def _prep_core_inputs(b, g, item, timestamp, correct, Qm, c_embed, d_embed, f_embed,
                      mu_q, r_embed, Wd, bd, weights):
    f32 = np.float32
    bf = ml_dtypes.bfloat16
    it = item[b].astype(np.int64) - 1
    valid = it >= 0
    concept = np.where(valid[:, None], Qm[np.clip(it, 0, None)].astype(f32), 0.0)  # [S,C]
    ct2 = _chunked(np.ascontiguousarray(concept.T)).astype(bf)                      # [128, 2*S]
    cn = concept.sum(1)
    ccn = cn * correct[b].astype(f32)

    ts = timestamp[b].astype(np.float64)
    dtm = ts[:, None] - ts[None, :]                       # [S, S]
    tdt = np.zeros((128, 3 * NB, S), f32)
    for pi, p in enumerate("qkr"):
        th2 = float(np.asarray(weights[p + "_theta"], np.float64)[0, 0]) ** 2
        for ib in range(NB):
            tdt[:, pi * NB + ib, :] = (th2 * dtm[128 * ib:128 * (ib + 1), :]).astype(f32)

    r, c = np.mgrid[0:128, 0:128]
    blob = np.zeros((128, 1536), f32)
    blob[:, BL_MASKNS:BL_MASKNS + 128] = np.where(c <= r, 0.0, NEG)
    blob[:, BL_MASKS:BL_MASKS + 128] = np.where(c < r, 0.0, NEG)
    blob[:, BL_I128:BL_I128 + 128] = np.eye(128, dtype=f32)
    blob[:, BL_CEMB:BL_CEMB + 512] = _chunked((mu_q * d_embed + c_embed).astype(f32))
    blob[:, BL_FEMB:BL_FEMB + 512] = _chunked((mu_q * f_embed).astype(f32))
    blob[:, BL_WDX:BL_WDX + ND] = Wd[D:2 * D].reshape(ND, 128).T
    misc = np.zeros((1, 2048), f32)
    misc[0, MI_R0:MI_R0 + D] = r_embed[0]
    misc[0, MI_DR:MI_DR + D] = r_embed[1] - r_embed[0]
    misc[0, MI_CN:MI_CN + S] = cn
    misc[0, MI_CCN:MI_CCN + S] = ccn
    misc_bf = misc.astype(bf)
    misc_bf[0, MI_BD:MI_BD + 2] = (
        np.asarray(bd, f32).reshape(-1)[:1].view(np.uint16).view(bf))

    hs = range(HPC * g, HPC * g + HPC)

    def phase_w(p):
        wQ, wK, wV, wO = (weights[p + "_wQ"], weights[p + "_wK"],
                          weights[p + "_wV"], weights[p + "_wO"])
        Ms = [_chunked((wQ[h] @ wK[h].T / np.sqrt(f32(D))).astype(f32)) for h in hs]
        Us = [(wV[h] @ wO[h * D:(h + 1) * D]).astype(f32) for h in hs]
        return Ms, Us

    qM, qU = phase_w("q")
    kM, kU = phase_w("k")
    rM, rU = phase_w("r")
    wdo = Wd[:D].reshape(D, 1).astype(f32)
    qw = np.concatenate(qM + [_chunked(u) for u in qU], axis=1)
    kw = np.concatenate(kM + [_chunked(u) for u in kU], axis=1)
    ruv = [_chunked(u @ wdo) for u in rU]                  # each [128, 2]
    rw = np.concatenate(rM + ruv, axis=1)

    return {
        "ct2": ct2,
        "blob": blob.astype(bf),
        "misc": misc_bf,
        "tdt": tdt.reshape(128, 3 * NB, S).astype(bf),
        "qw": qw.astype(bf),
        "kw": kw.astype(bf),
        "rw": rw.astype(bf),
    }


LAST_RESULTS = [None]


def kernel(item, timestamp, correct, Qm, c_embed, d_embed, f_embed, mu_q,
           r_embed, Wd, bd, q_wQ, q_wK, q_wV, q_wO, q_theta,
           k_wQ, k_wK, k_wV, k_wO, k_theta, r_wQ, r_wK, r_wV, r_wO, r_theta,
           _debug=False, _trace=False):
    weights = {
        "q_wQ": q_wQ, "q_wK": q_wK, "q_wV": q_wV, "q_wO": q_wO, "q_theta": q_theta,
        "k_wQ": k_wQ, "k_wK": k_wK, "k_wV": k_wV, "k_wO": k_wO, "k_theta": k_theta,
        "r_wQ": r_wQ, "r_wK": r_wK, "r_wV": r_wV, "r_wO": r_wO, "r_theta": r_theta,
    }
    weights = {k: np.asarray(v) for k, v in weights.items()}
    args = (np.asarray(item), np.asarray(timestamp), np.asarray(correct),
            np.asarray(Qm), np.asarray(c_embed), np.asarray(d_embed),
            np.asarray(f_embed), np.asarray(mu_q), np.asarray(r_embed),
            np.asarray(Wd), np.asarray(bd))
    in_maps = []
    for core in range(N_CORES):
        b, g = core // 2, core % 2
        in_maps.append(_prep_core_inputs(b, g, *args, weights))
    nc = _get_nc(_debug)
    res = run_bass_kernel_spmd(nc, in_maps, core_ids=list(range(N_CORES)),
                               trace=_trace,
                               trace_cores=list(range(N_CORES)) if _trace == "all" else None)
    LAST_RESULTS[0] = res
    outs = res.results
    pred = np.zeros((B, S, 1), np.float32)
    for b in range(B):
        pred[b, :, 0] = outs[2 * b]["out"][0]
    if _debug:
        return pred, outs
    return pred


# revision 19
# speedup vs baseline: 1.9965x; 1.0004x over previous
"""AKT (attentive knowledge tracing) forward pass on 8 TRN2 NeuronCores.

Sharding: batch b = core//2 across 4 core-pairs; within a pair, the 8 heads
of each of the 3 MHA blocks are split 4+4 (core%2).  Pairwise AllReduces
merge the head-partial wO outputs of mha-q and mha-k; the mha-r output is
only ever consumed through Wd[:D], so wO_r @ wd_o is folded per head on the
host (rank-1 AV) and a final [1,S] logit AllReduce merges the pair.

Host preps conceptT (the Qm gather), rhs2 (=[cn, cn*correct]), and
tdt = theta^2 * (t_i - t_j) per phase, so the device never gathers or
builds dt.  Device-side per core:
  xT = cemb2^T conceptT ; yT = femb2^T conceptT + r0/dr x rhs2
  phase q (4 heads), AR(x) overlapped with phase k (4 heads), AR(y)
  overlapped with phase r's scores+softmax (which need only x_hat),
  then rank-1 AV against y_hat, logit AR, sigmoid.

Per-block softmax chain engine split:
  ACT: e=exp(sc), f=exp(arg), e2=exp(s)+accum
  DVE: cs=scan(e), recips, nd=cs/r-1, s=sc*f
  GPS: arg=nd*tdt, A=e2/r2
"""

import os
import numpy as np
import ml_dtypes

import concourse.bass as bass
import concourse.mybir as mybir
from concourse import bacc, tile
from concourse.bass_utils import run_bass_kernel_spmd

F32 = mybir.dt.float32
BF16 = mybir.dt.bfloat16
AF = mybir.ActivationFunctionType
OP = mybir.AluOpType

B, S, P, C, D, H = 4, 512, 5000, 256, 256, 8
NB = S // 128           # 4 row blocks
ND = D // 128           # 2 chunks of D
HPC = H // 2            # heads per core
N_CORES = 8
NEG = -30000.0

# blob layout (bf16 [128, 1536])
BL_MASKNS = 0
BL_MASKS = 128
BL_I128 = 256
BL_CEMB = 384
BL_FEMB = 896
BL_WDX = 1408
# misc layout (bf16 [1, 2048]): r0 | dr | cn | ccn | bd(f32 as 2 slots)
MI_R0, MI_DR, MI_CN, MI_CCN, MI_BD = 0, 256, 512, 1024, 1536


def build_kernel(debug=False):
    nc = bacc.Bacc(None, target_bir_lowering=False, debug=False, num_devices=N_CORES)

    dp = lambda name, shape, dt: nc.declare_dram_parameter(name, shape, dt, isOutput=False)
    ct2_d = dp("ct2", [128, ND * S], BF16)
    blob_d = dp("blob", [128, 1536], BF16)
    misc_d = dp("misc", [1, 2048], BF16)
    tdt_d = dp("tdt", [128, 3 * NB, S], BF16)
    qw_d = dp("qw", [128, 2 * HPC * 512], BF16)    # m chunks then wu chunks
    kw_d = dp("kw", [128, 2 * HPC * 512], BF16)
    rw_d = dp("rw", [128, HPC * 512 + HPC * ND], BF16)  # m chunks then uvec chunks

    out_d = nc.declare_dram_parameter("out", [1, S], F32, isOutput=True)
    dbg_d = {}
    if debug:
        for name in ("xT", "yT"):
            dbg_d[name] = nc.declare_dram_parameter("dbg_" + name, [D, S], BF16, isOutput=True)
        for name in ("xhatT", "yhatT"):
            dbg_d[name] = nc.declare_dram_parameter("dbg_" + name, [D, S], BF16, isOutput=True)

    from contextlib import ExitStack
    with tile.TileContext(nc) as tc, ExitStack() as es:
        pp_o = es.enter_context(tc.tile_pool(name="pp_o", bufs=2, space="PSUM"))
        pp_sc = es.enter_context(tc.tile_pool(name="pp_sc", bufs=3, space="PSUM"))
        pp_w = es.enter_context(tc.tile_pool(name="pp_w", bufs=3, space="PSUM"))
        wk = es.enter_context(tc.tile_pool(name="wk", bufs=3))
        hp = es.enter_context(tc.tile_pool(name="hp", bufs=3))
        pers = es.enter_context(tc.tile_pool(name="pers", bufs=1))
        dram = es.enter_context(tc.tile_pool(name="dram", bufs=2, space="DRAM"))

        pt = lambda shape, dt, name: pers.tile(shape, dt, name=name, tag=name)

        # ---------------- persistent SBUF + loads ---------------------------
        cT = pt([128, ND, S], BF16, "cT")
        blob = pt([128, 1536], BF16, "blob")
        misc = pt([1, 2048], BF16, "misc")
        tdt = pt([128, 3 * NB, S], BF16, "tdt")
        qw = pt([128, 2 * HPC, 512], BF16, "qw")
        kw = pt([128, 2 * HPC, 512], BF16, "kw")
        rwm = pt([128, HPC, 512], BF16, "rwm")
        rwu = pt([128, HPC, ND], BF16, "rwu")

        xT = [pt([128, S], BF16, f"xT{a}") for a in range(ND)]
        yT = [pt([128, S], BF16, f"yT{a}") for a in range(ND)]
        xh_t = pt([128, ND, S], BF16, "xh")
        yh_t = pt([128, ND, S], BF16, "yh")
        xhT = [xh_t[:, a, :] for a in range(ND)]
        yhT = [yh_t[:, a, :] for a in range(ND)]

        # upfront DMAs: sync + vector queues, first-needed first
        nc.sync.dma_start(cT[:].rearrange("p a s -> p (a s)"), ct2_d[:])
        nc.scalar.dma_start(blob[:], blob_d[:])
        nc.scalar.dma_start(misc[:], misc_d[:])
        nc.sync.dma_start(qw[:].rearrange("p a s -> p (a s)"), qw_d[:])
        nc.scalar.dma_start(tdt[:].rearrange("p a s -> p (a s)"), tdt_d[:])
        nc.sync.dma_start(kw[:].rearrange("p a s -> p (a s)"), kw_d[:])
        nc.scalar.dma_start(rwm[:].rearrange("p a s -> p (a s)"), rw_d[:, :HPC * 512])
        nc.scalar.dma_start(rwu[:].rearrange("p a s -> p (a s)"), rw_d[:, HPC * 512:])

        maskns = blob[:, BL_MASKNS:BL_MASKNS + 128]
        masks = blob[:, BL_MASKS:BL_MASKS + 128]
        i128b = blob[:, BL_I128:BL_I128 + 128]
        cemb = blob[:, BL_CEMB:BL_CEMB + 512]
        femb = blob[:, BL_FEMB:BL_FEMB + 512]
        wdx = blob[:, BL_WDX:BL_WDX + ND]
        r0v = misc[:, MI_R0:MI_R0 + D]
        drv = misc[:, MI_DR:MI_DR + D]
        cnr = misc[:, MI_CN:MI_CN + S]
        ccnr = misc[:, MI_CCN:MI_CCN + S]
        bdv = misc[:, MI_BD:MI_BD + 2].bitcast(F32)

        # ---------------- embedding ----------------------------------------
        for ec in range(ND):
            x_ps = pp_w.tile([128, S], F32, name="x_ps", tag="w")
            for a in range(ND):
                nc.tensor.matmul(x_ps[:], cemb[:, a * D + 128 * ec: a * D + 128 * (ec + 1)],
                                 cT[:, a, :], start=(a == 0), stop=(a == ND - 1))
            nc.scalar.copy(xT[ec][:], x_ps[:])
            y_ps = pp_w.tile([128, S], F32, name="y_ps", tag="w")
            for a in range(ND):
                nc.tensor.matmul(y_ps[:], femb[:, a * D + 128 * ec: a * D + 128 * (ec + 1)],
                                 cT[:, a, :], start=(a == 0), stop=False)
            nc.tensor.matmul(y_ps[:], r0v[:, 128 * ec:128 * (ec + 1)], cnr[:],
                             start=False, stop=False)
            nc.tensor.matmul(y_ps[:], drv[:, 128 * ec:128 * (ec + 1)], ccnr[:],
                             start=False, stop=True)
            nc.vector.tensor_copy(yT[ec][:], y_ps[:])
        if debug:
            for a in range(ND):
                nc.sync.dma_start(dbg_d["xT"][128 * a:128 * (a + 1), :], xT[a][:])
                nc.sync.dma_start(dbg_d["yT"][128 * a:128 * (a + 1), :], yT[a][:])

        # ---------------- softmax chain, software-pipelined in 2 stages -----
        def chain_s1(sc_ps, tdt_row, J, strict0, a_dst):
            e = wk.tile([128, S], BF16, name="e", tag="e")
            nc.scalar.activation(e[:, :J], sc_ps[:, :J], AF.Exp)
            cs = wk.tile([128, S], BF16, name="cs", tag="cs")
            nc.vector.tensor_tensor_scan(cs[:, :J], e[:, :J], e[:, :J],
                                         0.0, OP.add, OP.bypass)
            rec = wk.tile([128, 1], F32, name="rec", tag="rec")
            if strict0:
                rr = wk.tile([128, 1], F32, name="rr", tag="rr")
                nc.vector.tensor_scalar_max(rr[:], cs[:, J - 1:J], 1e-30)
                nc.vector.reciprocal(rec[:], rr[:])
            else:
                nc.vector.reciprocal(rec[:], cs[:, J - 1:J])
            nd = wk.tile([128, S], BF16, name="nd", tag="nd")
            nc.vector.tensor_scalar(nd[:, :J], cs[:, :J], rec[:], -1.0, OP.mult, OP.add)
            arg = wk.tile([128, S], BF16, name="arg", tag="arg")
            nc.gpsimd.tensor_mul(arg[:, :J], nd[:, :J], tdt_row[:, :J])
            return (sc_ps, arg, J, strict0, a_dst)

        def chain_s2(st):
            sc_ps, arg, J, strict0, a_dst = st
            f = wk.tile([128, S], BF16, name="f", tag="f")
            nc.scalar.activation(f[:, :J], arg[:, :J], AF.Exp)
            s = wk.tile([128, S], BF16, name="s", tag="s")
            nc.vector.tensor_mul(s[:, :J], sc_ps[:, :J], f[:, :J])
            e2 = wk.tile([128, S], BF16, name="e2", tag="e2")
            r2 = wk.tile([128, 1], F32, name="r2", tag="r2")
            nc.scalar.activation(e2[:, :J], s[:, :J], AF.Exp, accum_out=r2[:])
            rec2 = wk.tile([128, 1], F32, name="rec2", tag="rec2")
            if strict0:
                nc.vector.tensor_scalar_max(r2[:], r2[:], 1e-30)
            nc.vector.reciprocal(rec2[:], r2[:])
            nc.vector.tensor_scalar_mul(a_dst[:, :J], e2[:, :J], rec2[:])

        pend = []

        def chain_push(st):
            pend.append(st)
            if len(pend) > 1:
                chain_s2(pend.pop(0))

        def chain_flush():
            while pend:
                chain_s2(pend.pop(0))

        def scores_block(gt_sb, qxT, ib, mask):
            J = 128 * (ib + 1)
            sc_ps = pp_sc.tile([128, S], F32, name="sc_ps", tag="sc")
            for a in range(ND):
                nc.tensor.matmul(sc_ps[:, :J], gt_sb[a][:, 128 * ib:128 * (ib + 1)],
                                 qxT[a][:, :J], start=(a == 0), stop=False)
            nc.tensor.matmul(sc_ps[:, 128 * ib:J], i128b, mask, start=False, stop=True)
            return sc_ps, J

        def gt_head(msl, qxT):
            """G^T = M^T qx^T, [ND] bf16 [128,S] tiles.  msl: [128,512] M chunks."""
            gt_sb = [hp.tile([128, S], BF16, name=f"gt{ec}", tag=f"gt{ec}")
                     for ec in range(ND)]
            for ec in range(ND):
                gt_ps = pp_w.tile([128, S], F32, name="gt_ps", tag="w")
                for a in range(ND):
                    nc.tensor.matmul(gt_ps[:],
                                     msl[:, a * D + 128 * ec: a * D + 128 * (ec + 1)],
                                     qxT[a][:], start=(a == 0), stop=(a == ND - 1))
                if ec == 0:
                    nc.scalar.copy(gt_sb[ec][:], gt_ps[:])
                else:
                    nc.vector.tensor_copy(gt_sb[ec][:], gt_ps[:])
            return gt_sb

        # ---------------- full MHA phase (q / k) ----------------------------
        def mha_phase(wsl, qxT, vxT, tdt_base):
            o_ps = [pp_o.tile([128, S], F32, name=f"o_ps{ec}", tag="o")
                    for ec in range(ND)]
            for h in range(HPC):
                # WV[j, e] = sum_d vx[j, d] wU[d, e]
                wu = wsl[:, HPC + h, :]
                wv_sb = [hp.tile([128, S], BF16, name=f"wv{q_}", tag=f"wv{q_}")
                         for q_ in range(2)]
                for half in range(2):
                    wv_ps = pp_w.tile([128, S], F32, name="wv_ps", tag="w")
                    for j2 in range(2):
                        jb = 2 * half + j2
                        for a in range(ND):
                            nc.tensor.matmul(
                                wv_ps[:, 256 * j2:256 * (j2 + 1)],
                                vxT[a][:, 128 * jb:128 * (jb + 1)],
                                wu[:, a * D:(a + 1) * D],
                                start=(a == 0), stop=(a == ND - 1))
                    if half == 0:
                        nc.scalar.copy(wv_sb[half][:], wv_ps[:])
                    else:
                        nc.vector.tensor_copy(wv_sb[half][:], wv_ps[:])

                gt_sb = gt_head(wsl[:, h, :], qxT)

                a_full = hp.tile([128, NB, S], BF16, name="a_full", tag="af")
                if os.environ.get("AKT_SIM"):
                    nc.gpsimd.memset(a_full[:], 0)
                for ib in range(NB):
                    sc_ps, J = scores_block(gt_sb, qxT, ib, maskns)
                    chain_push(chain_s1(sc_ps, tdt[:, tdt_base + ib, :], J, False,
                                        a_full[:, ib, :]))
                chain_flush()

                e2t = hp.tile([128, NB * NB, 128], BF16, name="e2t", tag="e2t")
                nc.sync.dma_start_transpose(e2t[:], a_full[:].rearrange("p a j -> p (a j)"))
                for ec in range(ND):
                    for jb in range(NB):
                        rhs = e2t[:, NB * jb + jb:NB * NB:NB, :]
                        lhsT = wv_sb[jb // 2][:, 256 * (jb % 2) + 128 * ec:
                                              256 * (jb % 2) + 128 * (ec + 1)]
                        nc.tensor.matmul(o_ps[ec][:, 128 * jb:], lhsT, rhs,
                                         start=(h == 0 and jb == 0),
                                         stop=(h == HPC - 1 and jb == NB - 1),
                                         skip_group_check=True)
            return o_ps

        def reduce_pair(o_ps, dst_t, name):
            """o_ps [ND] psum -> bf16 -> DRAM -> pair AllReduce; fetch deferred."""
            part = pt([128, ND, S], BF16, f"{name}part")
            nc.scalar.copy(part[:, 0, :], o_ps[0][:])
            nc.vector.tensor_copy(part[:, 1, :], o_ps[1][:])
            bnc = dram.tile([128, ND * S], BF16, name=f"bnc_{name}")
            bnco = dram.tile([128, ND * S], BF16, name=f"bnco_{name}")
            nc.sync.dma_start(bnc[:], part[:].rearrange("p a s -> p (a s)"))
            nc.gpsimd.collective_compute(
                "AllReduce", OP.add,
                replica_groups=[[0, 1], [2, 3], [4, 5], [6, 7]],
                ins=[bnc.opt()], outs=[bnco.opt()])

            def fetch():
                nc.sync.dma_start(dst_t[:].rearrange("p a s -> p (a s)"), bnco[:])
            return fetch

        # ---------------- phase q, AR(x) over phase k, AR(y) ----------------
        o_q = mha_phase(qw, xT, xT, 0)
        fetch_x = reduce_pair(o_q, xh_t, "x")
        o_k = mha_phase(kw, yT, yT, NB)
        fetch_x()
        fetch_y = reduce_pair(o_k, yh_t, "y")

        # ---------------- phase r: scores+softmax on x_hat only -------------
        e2t_r = [pt([128, NB * NB, 128], BF16, f"e2tr{h}") for h in range(HPC)]
        for h in range(HPC):
            gt_sb = gt_head(rwm[:, h, :], xhT)
            a_full = hp.tile([128, NB, S], BF16, name="a_full", tag="af")
            if os.environ.get("AKT_SIM"):
                nc.gpsimd.memset(a_full[:], 0)
            for ib in range(NB):
                sc_ps, J = scores_block(gt_sb, xhT, ib, masks)
                chain_push(chain_s1(sc_ps, tdt[:, 2 * NB + ib, :], J, ib == 0,
                                    a_full[:, ib, :]))
            chain_flush()
            nc.sync.dma_start_transpose(e2t_r[h][:],
                                        a_full[:].rearrange("p a j -> p (a j)"))
        fetch_y()
        if debug:
            for a in range(ND):
                nc.sync.dma_start(dbg_d["xhatT"][128 * a:128 * (a + 1), :], xhT[a][:])
                nc.sync.dma_start(dbg_d["yhatT"][128 * a:128 * (a + 1), :], yhT[a][:])

        # ---------------- phase r tail: rank-1 AV against y_hat -------------
        lgx_ps = pp_o.tile([1, S], F32, name="lgx_ps", tag="o")
        for a in range(ND):
            nc.tensor.matmul(lgx_ps[:], wdx[:, a:a + 1], xhT[a][:],
                             start=(a == 0), stop=(a == ND - 1))
        o3_ps = pp_o.tile([1, S], F32, name="o3_ps", tag="o")
        for h in range(HPC):
            wvl_ps = pp_w.tile([128, NB], F32, name="wvl_ps", tag="w")
            for jb in range(NB):
                for a in range(ND):
                    nc.tensor.matmul(wvl_ps[:, jb:jb + 1],
                                     yhT[a][:, 128 * jb:128 * (jb + 1)],
                                     rwu[:, h, a:a + 1],
                                     start=(a == 0), stop=(a == ND - 1))
            wvl = hp.tile([128, NB], BF16, name="wvl", tag="wvl")
            nc.vector.tensor_copy(wvl[:], wvl_ps[:])
            for jb in range(NB):
                for ib in range(jb, NB):
                    nc.tensor.matmul(o3_ps[0:1, 128 * ib:128 * (ib + 1)],
                                     wvl[:, jb:jb + 1],
                                     e2t_r[h][:, NB * ib + jb, :],
                                     start=(h == 0 and jb == 0 and ib == 0),
                                     stop=(h == HPC - 1 and jb == NB - 1 and ib == NB - 1),
                                     skip_group_check=True)

        # ---------------- logit AR + sigmoid --------------------------------
        lg3 = pt([1, S], F32, "lg3")
        nc.vector.tensor_copy(lg3[:], o3_ps[:])
        nc.gpsimd.memset(lg3[:, 0:1], 0)
        bnc2 = dram.tile([1, S], F32, name="bnc2")
        bnc2o = dram.tile([1, S], F32, name="bnc2o")
        nc.sync.dma_start(bnc2[:], lg3[:])
        nc.gpsimd.collective_compute(
            "AllReduce", OP.add,
            replica_groups=[[0, 1], [2, 3], [4, 5], [6, 7]],
            ins=[bnc2.opt()], outs=[bnc2o.opt()])
        lg3o = pt([1, S], F32, "lg3o")
        nc.sync.dma_start(lg3o[:], bnc2o[:])
        logit = pt([1, S], F32, "logit")
        nc.vector.tensor_add(logit[:], lgx_ps[:], lg3o[:])
        pred = pt([1, S], F32, "pred")
        nc.scalar.activation(pred[:], logit[:], AF.Sigmoid, bias=bdv)
        nc.sync.dma_start(out_d[:], pred[:])

    nc.finalize()
    return nc


# ---------------------------------------------------------------------------
_NC_CACHE = {}


def _get_nc(debug=False):
    if debug not in _NC_CACHE:
        _NC_CACHE[debug] = build_kernel(debug)
    return _NC_CACHE[debug]


def _chunked(w):
    """[n*128, M] -> [128, n*M] (chunk-major columns)."""
    n = w.shape[0] // 128
    return np.ascontiguousarray(
        w.reshape(n, 128, w.shape[1]).transpose(1, 0, 2).reshape(128, n * w.shape[1]))


def _prep_core_inputs(b, g, item, timestamp, correct, Qm, c_embed, d_embed, f_embed,
                      mu_q, r_embed, Wd, bd, weights):
    f32 = np.float32
    bf = ml_dtypes.bfloat16
    it = item[b].astype(np.int64) - 1
    valid = it >= 0
    concept = np.where(valid[:, None], Qm[np.clip(it, 0, None)].astype(f32), 0.0)  # [S,C]
    ct2 = _chunked(np.ascontiguousarray(concept.T)).astype(bf)                      # [128, 2*S]
    cn = concept.sum(1)
    ccn = cn * correct[b].astype(f32)

    ts = timestamp[b].astype(np.float64)
    dtm = ts[:, None] - ts[None, :]                       # [S, S]
    tdt = np.zeros((128, 3 * NB, S), f32)
    for pi, p in enumerate("qkr"):
        th2 = float(np.asarray(weights[p + "_theta"], np.float64)[0, 0]) ** 2
        for ib in range(NB):
            tdt[:, pi * NB + ib, :] = (th2 * dtm[128 * ib:128 * (ib + 1), :]).astype(f32)

    r, c = np.mgrid[0:128, 0:128]
    blob = np.zeros((128, 1536), f32)
    blob[:, BL_MASKNS:BL_MASKNS + 128] = np.where(c <= r, 0.0, NEG)
    blob[:, BL_MASKS:BL_MASKS + 128] = np.where(c < r, 0.0, NEG)
    blob[:, BL_I128:BL_I128 + 128] = np.eye(128, dtype=f32)
    blob[:, BL_CEMB:BL_CEMB + 512] = _chunked((mu_q * d_embed + c_embed).astype(f32))
    blob[:, BL_FEMB:BL_FEMB + 512] = _chunked((mu_q * f_embed).astype(f32))
    blob[:, BL_WDX:BL_WDX + ND] = Wd[D:2 * D].reshape(ND, 128).T
    misc = np.zeros((1, 2048), f32)
    misc[0, MI_R0:MI_R0 + D] = r_embed[0]
    misc[0, MI_DR:MI_DR + D] = r_embed[1] - r_embed[0]
    misc[0, MI_CN:MI_CN + S] = cn
    misc[0, MI_CCN:MI_CCN + S] = ccn
    misc_bf = misc.astype(bf)
    misc_bf[0, MI_BD:MI_BD + 2] = (
        np.asarray(bd, f32).reshape(-1)[:1].view(np.uint16).view(bf))

    hs = range(HPC * g, HPC * g + HPC)

    def phase_w(p):
        wQ, wK, wV, wO = (weights[p + "_wQ"], weights[p + "_wK"],
                          weights[p + "_wV"], weights[p + "_wO"])
        Ms = [_chunked((wQ[h] @ wK[h].T / np.sqrt(f32(D))).astype(f32)) for h in hs]
        Us = [(wV[h] @ wO[h * D:(h + 1) * D]).astype(f32) for h in hs]
        return Ms, Us

    qM, qU = phase_w("q")
    kM, kU = phase_w("k")
    rM, rU = phase_w("r")
    wdo = Wd[:D].reshape(D, 1).astype(f32)
    qw = np.concatenate(qM + [_chunked(u) for u in qU], axis=1)
    kw = np.concatenate(kM + [_chunked(u) for u in kU], axis=1)
    ruv = [_chunked(u @ wdo) for u in rU]                  # each [128, 2]
    rw = np.concatenate(rM + ruv, axis=1)

    return {
        "ct2": ct2,
        "blob": blob.astype(bf),
        "misc": misc_bf,
        "tdt": tdt.reshape(128, 3 * NB, S).astype(bf),
        "qw": qw.astype(bf),
        "kw": kw.astype(bf),
        "rw": rw.astype(bf),
    }


LAST_RESULTS = [None]


def kernel(item, timestamp, correct, Qm, c_embed, d_embed, f_embed, mu_q,
           r_embed, Wd, bd, q_wQ, q_wK, q_wV, q_wO, q_theta,
           k_wQ, k_wK, k_wV, k_wO, k_theta, r_wQ, r_wK, r_wV, r_wO, r_theta,
           _debug=False, _trace=False):
    weights = {
        "q_wQ": q_wQ, "q_wK": q_wK, "q_wV": q_wV, "q_wO": q_wO, "q_theta": q_theta,
        "k_wQ": k_wQ, "k_wK": k_wK, "k_wV": k_wV, "k_wO": k_wO, "k_theta": k_theta,
        "r_wQ": r_wQ, "r_wK": r_wK, "r_wV": r_wV, "r_wO": r_wO, "r_theta": r_theta,
    }
    weights = {k: np.asarray(v) for k, v in weights.items()}
    args = (np.asarray(item), np.asarray(timestamp), np.asarray(correct),
            np.asarray(Qm), np.asarray(c_embed), np.asarray(d_embed),
            np.asarray(f_embed), np.asarray(mu_q), np.asarray(r_embed),
            np.asarray(Wd), np.asarray(bd))
    in_maps = []
    for core in range(N_CORES):
        b, g = core // 2, core % 2
        in_maps.append(_prep_core_inputs(b, g, *args, weights))
    nc = _get_nc(_debug)
    res = run_bass_kernel_spmd(nc, in_maps, core_ids=list(range(N_CORES)),
                               trace=_trace,
                               trace_cores=list(range(N_CORES)) if _trace == "all" else None)
    LAST_RESULTS[0] = res
    outs = res.results
    pred = np.zeros((B, S, 1), np.float32)
    for b in range(B):
        pred[b, :, 0] = outs[2 * b]["out"][0]
    if _debug:
        return pred, outs
    return pred
